# revision 9
# baseline (speedup 1.0000x reference)
"""Trainium2 Bass kernel for nn_NeuralSplineNetwork (neural spline flow log_prob).

Strategy:
  * Pure data parallel over 8 NeuronCores: batch 8192 -> 1024 per core,
    all flow parameters replicated.
  * Feature-major ("transposed") layout on device: activations stored as
    [feature, batch]; every matmul uses the weight matrix [in, out] directly
    as the stationary lhsT operand, so no transposes anywhere.
  * Host-side prep: MADE masks folded into weights, hidden units permuted by
    degree so masked weights become block-triangular -> structurally-zero
    128x128 K-tiles are skipped on the PE.  Weights shipped in bf16.
  * Rational-quadratic spline evaluated plane-wise: softmax exps on ACT
    (bias/scale fused into the matmul epilogue), bin search via 7 monotone
    indicator compares, gathers via copy_predicated cascades (fp16), spline
    core fp32 on DVE with squares/logs on ACT (single activation table set:
    natural_log_exp_and_others).
"""

import os
from contextlib import ExitStack

import numpy as np
import ml_dtypes

D, H, NB, NBLOCKS, NFLOWS, B = 256, 512, 8, 2, 4, 8192
PM = 3 * NB - 1          # 23
TAIL = 3.0
MINW = MINH = MIND = 1e-3
SCALE = float(np.sqrt(H))
CB = float(np.log(np.exp(1.0 - MIND) - 1.0))   # raw derivative pad value
A_W = 1.0 - MINW * NB
TM2 = 2.0 * TAIL * MINW                         # 0.006
U_C = 2.0 * TAIL * A_W                          # u = U_C / S7
LOG2PI = float(np.log(2.0 * np.pi))

NCORES = 8
BLOC = B // NCORES       # 1024 batch per core
NHALF = 2
NCHUNK = 2
F = BLOC // NCHUNK       # 512 spline chunk width
NQ = PM * NHALF          # 46 final-matmul output planes


# --------------------------------------------------------------------------
# host-side weight preparation
# --------------------------------------------------------------------------

def _host_prep(inputs):
    in_deg = np.arange(1, D + 1)
    hid_deg = (np.arange(H) % (D - 1)) + 1
    out_deg = np.repeat(in_deg, PM)
    m0 = (in_deg[:, None] <= hid_deg[None, :]).astype(np.float32)
    mh = (hid_deg[:, None] <= hid_deg[None, :]).astype(np.float32)
    mf = (hid_deg[:, None] < out_deg[None, :]).astype(np.float32)

    perm = np.argsort(hid_deg, kind="stable")

    W0 = (np.asarray(inputs["W0"], np.float32) * m0[None])[:, :, perm]
    b0 = np.asarray(inputs["b0"], np.float32)[:, perm]
    Wa = (np.asarray(inputs["Wa"], np.float32) * mh[None, None])[:, :, perm][:, :, :, perm]
    ba = np.asarray(inputs["ba"], np.float32)[:, :, perm]
    Wb = (np.asarray(inputs["Wb"], np.float32) * mh[None, None])[:, :, perm][:, :, :, perm]
    bb = np.asarray(inputs["bb"], np.float32)[:, :, perm]
    Wf = (np.asarray(inputs["Wf"], np.float32) * mf[None])[:, perm, :]
    bf = np.asarray(inputs["bf"], np.float32)

    # Wf/bf plane-major column reorder: new col (pm*2+h)*128 + dl comes from
    # original col (h*128+dl)*PM + pm; fold 1/SCALE into pm<16 (uw, uh).
    cols = np.empty(D * PM, np.int64)
    colscale = np.empty(D * PM, np.float32)
    for pm in range(PM):
        for h in range(NHALF):
            q = pm * NHALF + h
            cols[q * 128:(q + 1) * 128] = (h * 128 + np.arange(128)) * PM + pm
            colscale[q * 128:(q + 1) * 128] = (1.0 / SCALE) if pm < 16 else 1.0
    Wfr = Wf[:, :, cols] * colscale[None, None]
    bfr = bf[:, cols] * colscale[None]

    rev = list(reversed(range(NFLOWS)))
    W0, b0, Wa, ba, Wb, bb, Wfr, bfr = (
        W0[rev], b0[rev], Wa[rev], ba[rev], Wb[rev], bb[rev], Wfr[rev], bfr[rev])

    BIAS = np.zeros((NFLOWS, 66, 128), np.float32)
    BIAS[:, 0:4] = b0.reshape(NFLOWS, 4, 128)
    BIAS[:, 4:8] = ba[:, 0].reshape(NFLOWS, 4, 128)
    BIAS[:, 8:12] = bb[:, 0].reshape(NFLOWS, 4, 128)
    BIAS[:, 12:16] = ba[:, 1].reshape(NFLOWS, 4, 128)
    BIAS[:, 16:20] = bb[:, 1].reshape(NFLOWS, 4, 128)
    BIAS[:, 20:66] = bfr.reshape(NFLOWS, NQ, 128)

    def active(Wmat, nk, nm):
        out = []
        for m in range(nm):
            ks = [k for k in range(nk)
                  if np.any(Wmat[k * 128:(k + 1) * 128, m * 128:(m + 1) * 128])]
            if not ks:
                ks = [0]
            assert ks == list(range(len(ks))), f"non-prefix K set {ks}"
            out.append(ks)
        return out

    act_k0 = active(W0[0], 2, 4)
    act_kh = active(Wa[0, 0], 4, 4)
    act_kf = active(Wfr[0], 4, NQ)

    tobf = lambda a: np.ascontiguousarray(a).astype(ml_dtypes.bfloat16)
    return dict(W0=tobf(W0), WA=tobf(Wa), WB=tobf(Wb), WF=tobf(Wfr),
                BIAS=np.ascontiguousarray(BIAS),
                act_k0=act_k0, act_kh=act_kh, act_kf=act_kf)


# --------------------------------------------------------------------------
# device kernel
# --------------------------------------------------------------------------

_CACHE = {}


def _build_nc(act_k0, act_kh, act_kf):
    import concourse.bass as bass
    import concourse.mybir as mybir
    import concourse.tile as tile
    from concourse import bacc

    dt = mybir.dt
    AF = mybir.ActivationFunctionType
    AL = mybir.AluOpType

    nc = bacc.Bacc("TRN2", target_bir_lowering=False, debug=False)

    xT = nc.dram_tensor("xT", [D, BLOC], dt.float32, kind="ExternalInput").ap()
    W0 = nc.dram_tensor("W0", [NFLOWS, D, H], dt.bfloat16, kind="ExternalInput").ap()
    WA = nc.dram_tensor("WA", [NFLOWS, NBLOCKS, H, H], dt.bfloat16, kind="ExternalInput").ap()
    WB = nc.dram_tensor("WB", [NFLOWS, NBLOCKS, H, H], dt.bfloat16, kind="ExternalInput").ap()
    WF = nc.dram_tensor("WF", [NFLOWS, H, D * PM], dt.bfloat16, kind="ExternalInput").ap()
    BIAS = nc.dram_tensor("BIAS", [NFLOWS, 66, 128], dt.float32, kind="ExternalInput").ap()
    OUT = nc.dram_tensor("logq", [1, BLOC], dt.float32, kind="ExternalOutput").ap()

    with tile.TileContext(nc) as tc, ExitStack() as ctx:
        consts = ctx.enter_context(tc.tile_pool(name="consts", bufs=1))
        zpool = ctx.enter_context(tc.tile_pool(name="zp", bufs=2))
        hpool = ctx.enter_context(tc.tile_pool(name="hp", bufs=2))
        tpool = ctx.enter_context(tc.tile_pool(name="tp", bufs=2))
        pppool = ctx.enter_context(tc.tile_pool(name="pp", bufs=2))
        wpool = ctx.enter_context(tc.tile_pool(name="wp", bufs=3))
        bpool = ctx.enter_context(tc.tile_pool(name="bp", bufs=2))
        s16 = ctx.enter_context(tc.tile_pool(name="s16", bufs=2))
        q16 = ctx.enter_context(tc.tile_pool(name="q16", bufs=10))
        f32p = ctx.enter_context(tc.tile_pool(name="f32p", bufs=14))
        opool = ctx.enter_context(tc.tile_pool(name="op", bufs=1))
        mmp = ctx.enter_context(tc.tile_pool(name="mmp", bufs=3, space="PSUM"))
        ldp = ctx.enter_context(tc.tile_pool(name="ldp", bufs=1, space="PSUM"))

        # ---- constants ----
        cb16 = consts.tile([128, F], dt.float16, tag="cb16")
        nc.gpsimd.memset(cb16, CB)
        onesc = consts.tile([128, 1], dt.bfloat16, tag="onesc")
        nc.gpsimd.memset(onesc, 1.0)
        nhalfc = consts.tile([128, 1], dt.bfloat16, tag="nhalfc")
        nc.gpsimd.memset(nhalfc, -0.5)

        ldacc = ldp.tile([1, BLOC], dt.float32, tag="ldacc")
        ld_started = [False] * NCHUNK

        def ld_accum(rhs_ap, lhs_tile, c0, c1, last=False):
            i = c0 // F
            nc.tensor.matmul(ldacc[:, c0:c1], lhs_tile, rhs_ap,
                             start=not ld_started[i], stop=last)
            ld_started[i] = True

        # ---- initial z (fp32 for spline) and z16 (bf16 for matmuls) ----
        z = zpool.tile([128, NHALF, BLOC], dt.float32, tag="z")
        nc.sync.dma_start(out=z[:, 0, :], in_=xT[0:128, :])
        nc.sync.dma_start(out=z[:, 1, :], in_=xT[128:256, :])
        z16 = zpool.tile([128, NHALF, BLOC], dt.bfloat16, tag="z16")
        nc.vector.tensor_copy(z16[:, 0, :], z[:, 0, :])
        nc.vector.tensor_copy(z16[:, 1, :], z[:, 1, :])

        for fi in range(NFLOWS):
            # ---------------- biases ----------------
            bias = bpool.tile([128, 66], dt.float32, tag="bias")
            nc.sync.dma_start(
                out=bias,
                in_=bass.AP(BIAS.tensor, BIAS.offset + fi * 66 * 128,
                            [[1, 128], [128, 66]]))

            def bias_col(i):
                return bias[:, i:i + 1]

            # ---------------- MADE layer 0: h = z @ W0 + b0 --------------
            h = hpool.tile([128, 4, BLOC], dt.bfloat16, tag="h")
            for m in range(4):
                ks = act_k0[m]
                w0t = wpool.tile([128, 2, 128], dt.bfloat16, tag="w0")
                nc.sync.dma_start(
                    out=w0t[:, 0:len(ks), :],
                    in_=bass.AP(W0.tensor, W0.offset + fi * D * H + m * 128,
                                [[H, 128], [128 * H, len(ks)], [1, 128]]))
                ps = mmp.tile([128, BLOC], dt.float32, tag="mm")
                for n0 in range(0, BLOC, 512):
                    for ki, k in enumerate(ks):
                        nc.tensor.matmul(
                            ps[:, n0:n0 + 512],
                            w0t[:, k, :], z16[:, k, n0:n0 + 512],
                            start=(ki == 0), stop=(ki == len(ks) - 1))
                nc.scalar.activation(h[:, m, :], ps, AF.Identity,
                                     bias=bias_col(m), scale=1.0)

            # ---------------- residual blocks ----------------
            for blk in range(NBLOCKS):
                t0 = tpool.tile([128, 4, BLOC], dt.bfloat16, tag="t")
                for m in range(4):
                    nc.vector.tensor_scalar(out=t0[:, m, :], in0=h[:, m, :],
                                            scalar1=0.0, scalar2=None,
                                            op0=AL.max)
                t1 = tpool.tile([128, 4, BLOC], dt.bfloat16, tag="t")
                for m in range(4):
                    ks = act_kh[m]
                    wat = wpool.tile([128, 4, 128], dt.bfloat16, tag="wht")
                    nc.sync.dma_start(
                        out=wat[:, 0:len(ks), :],
                        in_=bass.AP(
                            WA.tensor,
                            WA.offset + (fi * NBLOCKS + blk) * H * H + m * 128,
                            [[H, 128], [128 * H, len(ks)], [1, 128]]))
                    ps = mmp.tile([128, BLOC], dt.float32, tag="mm")
                    for n0 in range(0, BLOC, 512):
                        for ki, k in enumerate(ks):
                            nc.tensor.matmul(
                                ps[:, n0:n0 + 512],
                                wat[:, k, :], t0[:, k, n0:n0 + 512],
                                start=(ki == 0), stop=(ki == len(ks) - 1))
                    nc.scalar.activation(t1[:, m, :], ps, AF.Relu,
                                         bias=bias_col(4 + blk * 8 + m), scale=1.0)
                hn = hpool.tile([128, 4, BLOC], dt.bfloat16, tag="h")
                for m in range(4):
                    ks = act_kh[m]
                    wbt = wpool.tile([128, 4, 128], dt.bfloat16, tag="wht")
                    nc.sync.dma_start(
                        out=wbt[:, 0:len(ks), :],
                        in_=bass.AP(
                            WB.tensor,
                            WB.offset + (fi * NBLOCKS + blk) * H * H + m * 128,
                            [[H, 128], [128 * H, len(ks)], [1, 128]]))
                    ps = mmp.tile([128, BLOC], dt.float32, tag="mm")
                    for n0 in range(0, BLOC, 512):
                        for ki, k in enumerate(ks):
                            nc.tensor.matmul(
                                ps[:, n0:n0 + 512],
                                wbt[:, k, :], t1[:, k, n0:n0 + 512],
                                start=(ki == 0), stop=(ki == len(ks) - 1))
                    # hn = (ps + bb) + h
                    nc.vector.scalar_tensor_tensor(
                        out=hn[:, m, :], in0=ps, scalar=bias_col(8 + blk * 8 + m),
                        in1=h[:, m, :], op0=AL.add, op1=AL.add)
                h = hn

            # ---------------- final layer -> spline planes ----------------
            znew = zpool.tile([128, NHALF, BLOC], dt.float32, tag="z")
            znew16 = zpool.tile([128, NHALF, BLOC], dt.bfloat16, tag="z16")

            for half in range(NHALF):
                PP = [pppool.tile([128, PM, F], dt.float16, tag="ppc",
                                  name=f"ppc{cc}") for cc in range(NCHUNK)]
                for pm in range(PM):
                    q = pm * NHALF + half
                    ks = act_kf[q]
                    wft = wpool.tile([128, 4, 128], dt.bfloat16, tag="wf")
                    nc.sync.dma_start(
                        out=wft[:, 0:len(ks), :],
                        in_=bass.AP(
                            WF.tensor, WF.offset + fi * H * D * PM + q * 128,
                            [[D * PM, 128], [128 * D * PM, len(ks)], [1, 128]]))
                    ps = mmp.tile([128, BLOC], dt.float32, tag="mm")
                    for n0 in range(0, BLOC, 512):
                        for ki, k in enumerate(ks):
                            nc.tensor.matmul(
                                ps[:, n0:n0 + 512],
                                wft[:, k, :], h[:, k, n0:n0 + 512],
                                start=(ki == 0), stop=(ki == len(ks) - 1))
                    func = AF.Exp if pm < 16 else AF.Identity
                    for c in range(NCHUNK):
                        nc.scalar.activation(
                            PP[c][:, pm, :], ps[:, c * F:(c + 1) * F], func,
                            bias=bias_col(20 + q), scale=1.0)

                # ---------------- spline on (half, chunk) ----------------
                for c in range(NCHUNK):
                    pp = PP[c]
                    EUW = pp[:, 0:8, :]
                    EUH = pp[:, 8:16, :]
                    UD = pp[:, 16:23, :]
                    zs = z[:, half, c * F:(c + 1) * F]

                    def t32(nm="tmp"):
                        return f32p.tile([128, F], dt.float32, tag="f32",
                                         name=nm)

                    # xc / masks
                    xc = t32()
                    nc.gpsimd.tensor_scalar(out=xc, in0=zs, scalar1=-TAIL,
                                            scalar2=TAIL, op0=AL.max, op1=AL.min)
                    xcT = t32()
                    nc.gpsimd.tensor_scalar(out=xcT, in0=xc, scalar1=TAIL,
                                            scalar2=None, op0=AL.add)
                    ins = t32()
                    nc.vector.tensor_tensor(out=ins, in0=zs, in1=xc,
                                            op=AL.is_equal)
                    omask = q16.tile([128, F], dt.int16, tag="om", bufs=2)
                    nc.vector.scalar_tensor_tensor(
                        out=omask, in0=zs, scalar=0.0, in1=xc,
                        op0=AL.add, op1=AL.not_equal)
                    xc16 = q16.tile([128, F], dt.float16, tag="q")
                    nc.vector.tensor_copy(xc16, xc)

                    # cumsums (fp16)
                    S = s16.tile([128, 8, F], dt.float16, tag="s")
                    Sh = s16.tile([128, 8, F], dt.float16, tag="s")
                    nc.vector.tensor_copy(S[:, 0, :], EUW[:, 0, :])
                    nc.vector.tensor_copy(Sh[:, 0, :], EUH[:, 0, :])
                    for j in range(1, 8):
                        nc.vector.tensor_add(S[:, j, :], S[:, j - 1, :], EUW[:, j, :])
                        nc.vector.tensor_add(Sh[:, j, :], Sh[:, j - 1, :], EUH[:, j, :])

                    # u = U_C / S7
                    s7f = t32()
                    nc.vector.tensor_copy(s7f, S[:, 7, :])
                    scr = t32()
                    r8w = t32()
                    nc.vector.reciprocal_approx_accurate(r8w, s7f, scr)
                    uw = t32()
                    nc.vector.tensor_scalar(out=uw, in0=r8w, scalar1=U_C,
                                            scalar2=None, op0=AL.mult)
                    sh7f = t32()
                    nc.vector.tensor_copy(sh7f, Sh[:, 7, :])
                    r8h = t32()
                    nc.vector.reciprocal_approx_accurate(r8h, sh7f, scr)
                    uh = t32()
                    nc.vector.tensor_scalar(out=uh, in0=r8h, scalar1=U_C,
                                            scalar2=None, op0=AL.mult)
                    uw16 = q16.tile([128, F], dt.float16, tag="q")
                    nc.vector.tensor_copy(uw16, uw)

                    # indicators b_j = [xc16 - c_j >= uw16*S_{j-1}], j=1..7
                    bj = s16.tile([128, 7, F], dt.int16, tag="b")
                    uS = s16.tile([128, 7, F], dt.float16, tag="us")
                    for j in range(1, 8):
                        nc.vector.tensor_mul(uS[:, j - 1, :], uw16, S[:, j - 1, :])
                        nc.vector.scalar_tensor_tensor(
                            out=bj[:, j - 1, :], in0=xc16,
                            scalar=-(TM2 * j - TAIL), in1=uS[:, j - 1, :],
                            op0=AL.add, op1=AL.is_ge)

                    # idx = sum b_j
                    idx = q16.tile([128, F], dt.float16, tag="q")
                    nc.vector.tensor_add(idx, bj[:, 0, :], bj[:, 1, :])
                    for j in range(2, 7):
                        nc.vector.tensor_add(idx, idx, bj[:, j, :])

                    # gathers via copy_predicated cascades (fp16)
                    def casc(init_ap, planes):
                        g = q16.tile([128, F], dt.float16, tag="q")
                        if init_ap is None:
                            nc.vector.memset(g, 0.0)
                        else:
                            nc.vector.tensor_copy(g, init_ap)
                        for j in range(1, 8):
                            nc.vector.copy_predicated(g, bj[:, j - 1, :],
                                                      planes[j - 1])
                        return g

                    Q1 = casc(None, [S[:, j - 1, :] for j in range(1, 8)])
                    Q2 = casc(EUW[:, 0, :], [EUW[:, j, :] for j in range(1, 8)])
                    Q3 = casc(None, [Sh[:, j - 1, :] for j in range(1, 8)])
                    Q4 = casc(EUH[:, 0, :], [EUH[:, j, :] for j in range(1, 8)])
                    Q5 = casc(cb16, [UD[:, j - 1, :] for j in range(1, 8)])
                    Q6 = casc(UD[:, 0, :],
                              [UD[:, j, :] for j in range(1, 7)] + [cb16])

                    # d = MIND + ln(1 + exp(Q))   (exp/ln share one table set)
                    d0 = t32()
                    nc.scalar.activation(d0, Q5, AF.Exp)
                    nc.scalar.activation(d0, d0, AF.Ln, bias=1.0)
                    nc.gpsimd.tensor_scalar(out=d0, in0=d0, scalar1=MIND,
                                            scalar2=None, op0=AL.add)
                    d1 = t32()
                    nc.scalar.activation(d1, Q6, AF.Exp)
                    nc.scalar.activation(d1, d1, AF.Ln, bias=1.0)
                    nc.gpsimd.tensor_scalar(out=d1, in0=d1, scalar1=MIND,
                                            scalar2=None, op0=AL.add)

                    # ibw, ih, theta-num, ichT
                    ibw = t32()
                    nc.vector.tensor_mul(ibw, uw, Q2)
                    nc.gpsimd.tensor_scalar(out=ibw, in0=ibw, scalar1=TM2,
                                            scalar2=None, op0=AL.add)
                    ih = t32()
                    nc.vector.tensor_mul(ih, uh, Q4)
                    nc.gpsimd.tensor_scalar(out=ih, in0=ih, scalar1=TM2,
                                            scalar2=None, op0=AL.add)
                    thn = t32()
                    nc.vector.tensor_mul(thn, uw, Q1)
                    nc.vector.scalar_tensor_tensor(
                        out=thn, in0=idx, scalar=TM2, in1=thn,
                        op0=AL.mult, op1=AL.add)
                    nc.vector.tensor_tensor(out=thn, in0=xcT, in1=thn,
                                            op=AL.subtract)
                    ichT = t32()
                    nc.vector.tensor_mul(ichT, uh, Q3)
                    nc.vector.scalar_tensor_tensor(
                        out=ichT, in0=idx, scalar=TM2, in1=ichT,
                        op0=AL.mult, op1=AL.add)

                    ribw = t32()
                    nc.vector.reciprocal_approx_accurate(ribw, ibw, scr)
                    th = t32()
                    nc.vector.tensor_mul(th, thn, ribw)
                    dl = t32()
                    nc.vector.tensor_mul(dl, ih, ribw)

                    thsq = t32()
                    nc.scalar.activation(thsq, th, AF.Square)
                    t1m = t32()
                    nc.vector.tensor_tensor(out=t1m, in0=th, in1=thsq,
                                            op=AL.subtract)
                    d01 = t32()
                    nc.vector.tensor_add(d01, d0, d1)
                    s3 = t32()
                    nc.vector.scalar_tensor_tensor(
                        out=s3, in0=dl, scalar=-2.0, in1=d01,
                        op0=AL.mult, op1=AL.add)
                    den = t32()
                    nc.vector.tensor_mul(den, s3, t1m)
                    nc.vector.tensor_add(den, den, dl)
                    rden = t32()
                    nc.vector.reciprocal_approx_accurate(rden, den, scr)

                    # y = ichT + ih*(dl*thsq + d0*t1m)*rden - TAIL
                    acc = t32()
                    nc.vector.tensor_mul(acc, dl, thsq)
                    bb2 = t32()
                    nc.vector.tensor_mul(bb2, d0, t1m)
                    nc.vector.tensor_add(acc, acc, bb2)
                    nc.vector.tensor_mul(acc, acc, rden)
                    nc.vector.tensor_mul(acc, acc, ih)
                    nc.vector.tensor_add(acc, acc, ichT)
                    yz = znew[:, half, c * F:(c + 1) * F]
                    nc.vector.tensor_scalar(out=yz, in0=acc, scalar1=-TAIL,
                                            scalar2=None, op0=AL.add)
                    nc.vector.copy_predicated(yz, omask, zs)
                    if fi < NFLOWS - 1:
                        nc.vector.tensor_copy(
                            znew16[:, half, c * F:(c + 1) * F], yz)

                    # ld = ln(dnum * rden^2)
                    m1 = t32()
                    nc.vector.tensor_mul(m1, d1, thsq)
                    m2 = t32()
                    nc.vector.scalar_tensor_tensor(
                        out=m2, in0=dl, scalar=2.0, in1=t1m,
                        op0=AL.mult, op1=AL.mult)
                    nc.vector.tensor_add(m1, m1, m2)
                    omth = t32()
                    nc.gpsimd.tensor_scalar(out=omth, in0=th, scalar1=-1.0,
                                            scalar2=1.0, op0=AL.mult, op1=AL.add)
                    omsq = t32()
                    nc.scalar.activation(omsq, omth, AF.Square)
                    nc.vector.tensor_mul(omsq, omsq, d0)
                    nc.vector.tensor_add(m1, m1, omsq)
                    dlsq = t32()
                    nc.scalar.activation(dlsq, dl, AF.Square)
                    nc.vector.tensor_mul(m1, m1, dlsq)
                    r2 = t32()
                    nc.scalar.activation(r2, rden, AF.Square)
                    nc.vector.tensor_mul(m1, m1, r2)
                    ldt = t32()
                    nc.scalar.activation(ldt, m1, AF.Ln)
                    ldb = q16.tile([128, F], dt.bfloat16, tag="ldb", bufs=2)
                    nc.vector.tensor_tensor(out=ldb, in0=ldt, in1=ins,
                                            op=AL.mult)

                    ld_accum(ldb, onesc, c * F, (c + 1) * F)
            z = znew
            z16 = znew16

        # ---- base log prob: -0.5 * sum z^2 ----
        for half in range(NHALF):
            zsq = tpool.tile([128, BLOC], dt.bfloat16, tag="zsq")
            nc.scalar.activation(zsq, z[:, half, :], AF.Square)
            for c in range(NCHUNK):
                ld_accum(zsq[:, c * F:(c + 1) * F], nhalfc, c * F, (c + 1) * F,
                         last=(half == NHALF - 1))

        # ---- finalize ----
        outb = opool.tile([1, BLOC], dt.float32, tag="out")
        nc.vector.tensor_scalar(out=outb, in0=ldacc,
                                scalar1=-0.5 * D * LOG2PI, scalar2=None,
                                op0=AL.add)
        nc.sync.dma_start(out=OUT, in_=outb)

    nc.compile()
    return nc


def _get_compiled(prep):
    if "nc" not in _CACHE:
        _CACHE["nc"] = _build_nc(prep["act_k0"], prep["act_kh"], prep["act_kf"])
    return _CACHE["nc"]


# --------------------------------------------------------------------------
# entry point
# --------------------------------------------------------------------------

_LAST_RESULT = None


def kernel(**inputs) -> np.ndarray:
    global _LAST_RESULT
    from concourse.bass_utils import run_bass_kernel_spmd

    prep = _host_prep(inputs)
    nc = _get_compiled(prep)

    x = np.asarray(inputs["x"], np.float32)
    in_maps = []
    for c in range(NCORES):
        xs = np.ascontiguousarray(x[c * BLOC:(c + 1) * BLOC, :].T)
        in_maps.append({
            "xT": xs,
            "W0": prep["W0"],
            "WA": prep["WA"],
            "WB": prep["WB"],
            "WF": prep["WF"],
            "BIAS": prep["BIAS"],
        })
    res = run_bass_kernel_spmd(nc, in_maps, core_ids=list(range(NCORES)),
                               trace=os.environ.get("KTRACE", "0") == "1")
    _LAST_RESULT = res
    out = np.concatenate([np.asarray(res.results[c]["logq"]).reshape(BLOC)
                          for c in range(NCORES)])
    return out.astype(np.float32)
